# revision 32
# baseline (speedup 1.0000x reference)
"""Trainium2 Bass kernel for nn_AttentionBlock (B=16, C=512, H=W=32, 4 heads).

Strategy: data-parallel over batch across 8 NeuronCores (2 batch elements per
core), weights replicated, no collectives.  All matmuls in float32r (full PE
rate, ~1e-4 rounding).  Attention is computed in transposed score layout
scoresT[ks, qs] so that:
  - exp() runs on ScalarE straight out of PSUM (scale fused into activation),
  - softmax denominators come from a ones-vector matmul (PE, nearly free),
  - attn @ V contracts over the partition axis natively (no transposes),
  - the output projection consumes attn-out^T directly and the residual add
    happens in the natural [C, H*W] layout of x.

Weights are transposed on the host (input prep) so no on-device transposes
are needed.

uniform=True variant (gn_weight==1, gn_bias==0, which is what setup_inputs
produces): GroupNorm is the uniform affine (x-mean)*rstd, and because softmax
over ks is invariant to per-qs shifts, the whole normalization folds out of
the critical path:
  - QKV matmuls run on RAW x (f32r straight from DMA),
  - rstd^2 folds into the exp() scale (an AP),
  - the only surviving bias term (K-side, varying over ks) is a per-partition
    vector that exp()'s bias AP applies for free; it is produced by 8 tiny
    N=1 matmuls per head,
  - rstd on the V path folds into the softmax reciprocal,
  - the V bias is constant along ks, passes through the attention average
    unchanged, and folds into the output-projection bias via 4 tiny matmuls.
So Q/K/V PSUM->SBUF moves are PURE copies (ScalarE Identity), and the stats
chain (DVE-only, Newton rsqrt) has tens of microseconds of slack.

uniform=False: fully general fallback (materializes normalized xn).
"""

import numpy as np

import concourse.bacc as bacc
import concourse.bass as bass
import concourse.mybir as mybir
import concourse.tile as tile
from concourse.bass_utils import run_bass_kernel_spmd

# Problem constants (hardcoded per harness contract).
B = 16
C = 512
H = W = 32
S = H * W            # 1024
NH = 4               # heads
HD = C // NH         # 128
P = 128              # SBUF partitions
CT = C // P          # 4 channel tiles
ST = S // P          # 8 sequence tiles
N_CORES = 8
BPC = B // N_CORES   # 2 batch elements per core
EPS = 1e-5
SCALE = float(1.0 / np.sqrt(HD))

f32 = mybir.dt.float32
f32r = mybir.dt.float32r
ADD = mybir.AluOpType.add
MULT = mybir.AluOpType.mult
SUB = mybir.AluOpType.subtract
AF = mybir.ActivationFunctionType


def _build_nc(uniform):
    nc = bacc.Bacc("TRN2", target_bir_lowering=False)

    x_d = nc.dram_tensor("x", [BPC, C, S], f32r if uniform else f32,
                         kind="ExternalInput")
    # host passes w*.T (shape [c_in, c_out])
    w_d = {n: nc.dram_tensor(n, [C, C], f32r, kind="ExternalInput")
           for n in ("wq", "wk", "wv", "wo")}
    b_d = {n: nc.dram_tensor(n, [C], f32, kind="ExternalInput")
           for n in ("bq", "bk", "bv", "bo")}
    gnw_d = nc.dram_tensor("gn_weight", [C], f32, kind="ExternalInput")
    gnb_d = nc.dram_tensor("gn_bias", [C], f32, kind="ExternalInput")
    # host-packed small vectors in SBUF layout [P, n*CT]:
    # uniform: [bq, bk, gnw, gnb, bv, wqrs, wkrs, wvrs]; else [bq, bk, gnw, gnb]
    n_small = 8 if uniform else 4
    smalls_d = nc.dram_tensor("smalls", [P, n_small * CT], f32,
                              kind="ExternalInput")
    out_d = nc.dram_tensor("out", [BPC, C, S], f32, kind="ExternalOutput")

    x_view = x_d.rearrange("b (t p) s -> b p t s", p=P)
    out_view = out_d.rearrange("b (t p) s -> b p t s", p=P)

    with tile.TileContext(nc) as tc:
        with (
            tc.tile_pool(name="persist", bufs=1) as persist,
            tc.tile_pool(name="xn_pool", bufs=2) as xn_pool,
            tc.tile_pool(name="exp_pool", bufs=6) as exp_pool,
            tc.tile_pool(name="fin_pool", bufs=4) as fin_pool,
            tc.tile_pool(name="stat_pool", bufs=2) as stat_pool,
            tc.tile_pool(name="norm_pool", bufs=3) as norm_pool,
            tc.tile_pool(name="psum", bufs=1, space="PSUM") as psum,
        ):
            # ---------------- constants / small inputs ----------------
            ones_f32 = persist.tile([P, S // 2], f32)
            nc.vector.memset(ones_f32, 1.0)
            ones_col = persist.tile([P, 1], f32r)
            nc.vector.tensor_copy(ones_col, ones_f32[:, 0:1])
            ones_row = persist.tile([1, S // 2], f32r)
            nc.vector.tensor_copy(ones_row, ones_f32[0:1, :])

            smalls_sb = persist.tile([P, n_small * CT], f32)
            nc.sync.dma_start(smalls_sb, smalls_d[:, :])
            bq_sb = smalls_sb[:, 0 * CT:1 * CT]
            bk_sb = smalls_sb[:, 1 * CT:2 * CT]
            gnw_sb = smalls_sb[:, 2 * CT:3 * CT]
            gnb_sb = smalls_sb[:, 3 * CT:4 * CT]
            bo_row = persist.tile([1, C], f32r)
            nc.sync.dma_start(
                bo_row, b_d["bo"].rearrange("(o c) -> o c", o=1).bitcast(f32r))
            if uniform:
                bv_sb = smalls_sb[:, 4 * CT:5 * CT]
                wqrs_sb = smalls_sb[:, 5 * CT:6 * CT]
                wkrs_sb = smalls_sb[:, 6 * CT:7 * CT]
                wvrs_sb = smalls_sb[:, 7 * CT:8 * CT]
            else:
                bv_bcast = persist.tile([P, C], f32)
                nc.sync.dma_start(
                    bv_bcast,
                    bass.AP(tensor=b_d["bv"], offset=0, ap=[[0, P], [1, C]]),
                )

            # ---------------- x b0 first: per c-tile chunks ----------------
            x_sb = []
            for b in range(BPC):
                xb = persist.tile([P, CT, S], f32r if uniform else f32,
                                  name=f"x_sb{b}")
                x_sb.append(xb)
            wT = {n: persist.tile([P, CT, C], f32r, name=f"{n}T")
                  for n in ("wq", "wk", "wv", "wo")}
            w_views = {n: w_d[n].rearrange("(t p) o -> p t o", p=P)
                       for n in ("wq", "wk", "wv", "wo")}
            # interleave x_b0 and wq chunks: proj MM (ci=t) becomes ready
            # right as its pair lands; stats stream concurrently
            for t in range(CT):
                nc.sync.dma_start(x_sb[0][:, t], x_view[0][:, t])
                nc.sync.dma_start(wT["wq"][:, t], w_views["wq"][:, t])
            for name in ("wk", "wv", "wo"):
                nc.sync.dma_start(wT[name], w_views[name])
            nc.sync.dma_start(x_sb[1], x_view[1])
            if uniform:
                # exact-fp32 copy of x for the residual add (f32r reads are
                # rounded to ~12 mantissa bits by every engine)
                xres_sb = []
                for b in range(BPC):
                    xr = persist.tile([P, CT, S], f32, name=f"xres_sb{b}")
                    nc.sync.dma_start(xr, x_view[b].bitcast(f32))
                    xres_sb.append(xr)

            # persistent per-batch activation storage
            qT_sb = persist.tile([P, CT, S], f32r, name="qT_sb")
            kT_sb = persist.tile([P, CT, S], f32r, name="kT_sb")
            v_sb = persist.tile([P, ST, C], f32r, name="v_sb")
            outT_sb = persist.tile([P, CT, S], f32r, name="outT_sb")

            for b in range(BPC):
                # ------------- GroupNorm stats (off the critical path) ------
                stats6 = stat_pool.tile([P, CT * 2, 6], f32, tag="stats6")
                x_chunks = x_sb[b].rearrange("p t (u f) -> p (t u) f", f=512)
                for g in range(CT * 2):
                    nc.vector.bn_stats(stats6[:, g], x_chunks[:, g])
                mv = stat_pool.tile([P, 2], f32, tag="mv")
                nc.vector.bn_aggr(mv, stats6)
                msq = stat_pool.tile([P, 3], f32, tag="msq")
                nc.vector.tensor_copy(msq[:, 0:2], mv)
                nc.vector.tensor_tensor(msq[:, 2:3], mv[:, 0:1], mv[:, 0:1], MULT)
                # partition-sum via PE ones-matmul (fp32, tiny)
                red_ps = psum.tile([1, 4], f32, tag="sco", bufs=3)
                nc.tensor.matmul(red_ps[:, 0:3], ones_f32[:, 0:1], msq,
                                 start=True, stop=True)
                # sc: [mean, var+eps, avg_msq, mean^2, u, y, rstd, _]
                sc = stat_pool.tile([1, 8], f32, tag="sc")
                nc.vector.tensor_scalar_mul(sc[:, 0:3], red_ps[:, 0:3], 1.0 / P)
                nc.vector.tensor_tensor(sc[:, 3:4], sc[:, 0:1], sc[:, 0:1], MULT)
                nc.vector.tensor_tensor(sc[:, 1:2], sc[:, 1:2], sc[:, 2:3], ADD)
                nc.vector.tensor_tensor(sc[:, 1:2], sc[:, 1:2], sc[:, 3:4], SUB)
                nc.vector.tensor_scalar(sc[:, 1:2], sc[:, 1:2], EPS, None, ADD)
                u_t = sc[:, 4:5]
                nc.vector.reciprocal(u_t, sc[:, 1:2])      # u = 1/(var+eps)
                # y = rsqrt(u) = sqrt(var+eps), Newton from y0=1 (u ~= 1)
                y_t = sc[:, 5:6]
                nwt = stat_pool.tile([1, 1], f32, tag="nwt")
                nc.vector.tensor_copy(y_t, ones_f32[0:1, 0:1])
                for _ in range(3):
                    nc.vector.tensor_tensor(nwt, y_t, y_t, MULT)
                    nc.vector.tensor_tensor(nwt, nwt, u_t, MULT)
                    nc.vector.tensor_scalar(nwt, nwt, -0.5, 1.5, MULT, ADD)
                    nc.vector.tensor_tensor(y_t, y_t, nwt, MULT)
                rstd_t = sc[:, 6:7]
                nc.vector.tensor_tensor(rstd_t, u_t, y_t, MULT)

                if uniform:
                    # scal2 = [rstd, rstd*mean], broadcast via PE outer prod
                    scal2 = stat_pool.tile([1, 2], f32, tag="scal2")
                    nc.vector.tensor_copy(scal2[:, 0:1], rstd_t)
                    nc.vector.tensor_tensor(scal2[:, 1:2], rstd_t, sc[:, 0:1],
                                            MULT)
                    bc_ps = psum.tile([P, 2], f32, tag="sco", bufs=3)
                    nc.tensor.matmul(bc_ps, ones_f32[0:1, 0:P], scal2,
                                     start=True, stop=True)
                    bc = stat_pool.tile([P, 2], f32, tag="bc")
                    nc.vector.tensor_copy(bc, bc_ps)
                    rstd_c = bc[:, 0:1]
                    # b?p = bias - rstd*mean*rowsum(w)
                    bqp = stat_pool.tile([P, CT], f32, tag="bqp")
                    nc.vector.tensor_scalar(bqp, wqrs_sb, bc[:, 1:2], None, MULT)
                    nc.vector.tensor_tensor(bqp, bq_sb, bqp, SUB)
                    bkp = stat_pool.tile([P, CT], f32, tag="bkp")
                    nc.vector.tensor_scalar(bkp, wkrs_sb, bc[:, 1:2], None, MULT)
                    nc.vector.tensor_tensor(bkp, bk_sb, bkp, SUB)
                    # bvp = bv - rstd*mean*wvrs  (passes through attention)
                    tv = stat_pool.tile([P, CT], f32, tag="tv")
                    nc.vector.tensor_scalar(tv, wvrs_sb, bc[:, 1:2], None, MULT)
                    nc.vector.tensor_tensor(tv, bv_sb, tv, SUB)
                    bvp_r = stat_pool.tile([P, CT], f32r, tag="bvp_r")
                    nc.vector.tensor_copy(bvp_r, tv)
                    # delta_row[1, C] = sum_ci bvp_ci^T @ woT[ci]
                    pd = psum.tile([1, 512], f32, tag="sco", bufs=3)
                    for ci in range(CT):
                        nc.tensor.matmul(pd, bvp_r[:, ci:ci + 1],
                                         wT["wo"][:, ci, :],
                                         start=(ci == 0), stop=(ci == CT - 1))
                    bo2_row = stat_pool.tile([1, C], f32r, tag="bo2_row")
                    nc.vector.tensor_tensor(bo2_row, pd, bo_row, ADD)
                    proj_src = x_sb[b]
                else:
                    # general path: broadcast [mean, rstd]; A/Bc; xn
                    mr0 = stat_pool.tile([1, 2], f32, tag="mr0")
                    nc.vector.tensor_copy(mr0[:, 0:1], sc[:, 0:1])
                    nc.vector.tensor_copy(mr0[:, 1:2], rstd_t)
                    mr = stat_pool.tile([P, 2], f32, tag="mr")
                    nc.gpsimd.partition_broadcast(mr, mr0)
                    A = stat_pool.tile([P, CT], f32, tag="A")
                    nc.vector.tensor_scalar_mul(A, gnw_sb, mr[:, 1:2])
                    mA = stat_pool.tile([P, CT], f32, tag="mA")
                    nc.vector.tensor_scalar_mul(mA, A, mr[:, 0:1])
                    Bc = stat_pool.tile([P, CT], f32, tag="Bc")
                    nc.vector.tensor_tensor(Bc, gnb_sb, mA, SUB)
                    xn = xn_pool.tile([P, CT, S], f32r, tag="xn")
                    for t in range(CT):
                        nc.vector.tensor_scalar(
                            xn[:, t], x_sb[b][:, t], A[:, t:t + 1],
                            Bc[:, t:t + 1], MULT, ADD)
                    bo2_row = bo_row
                    proj_src = xn

                # ---------------- Q/K projections -> qT/kT [c_out, s] -------
                for (wname, dst, bias_t) in (("wq", qT_sb, bq_sb),
                                             ("wk", kT_sb, bk_sb)):
                    for co in range(CT):
                        for half in range(2):
                            sl = slice(half * 512, (half + 1) * 512)
                            pq = psum.tile([P, 512], f32, tag="acc", bufs=3)
                            for ci in range(CT):
                                nc.tensor.matmul(
                                    pq,
                                    wT[wname][:, ci, co * P:(co + 1) * P],
                                    proj_src[:, ci, sl],
                                    start=(ci == 0), stop=(ci == CT - 1))
                            if uniform:
                                bp = bqp if wname == "wq" else bkp
                                nc.vector.tensor_scalar(
                                    dst[:, co, sl], pq, rstd_c,
                                    bp[:, co:co + 1], MULT, ADD)
                            else:
                                nc.scalar.activation(
                                    dst[:, co, sl], pq, AF.Identity,
                                    bias=bias_t[:, co:co + 1], scale=1.0)

                # ---------------- V projection -> v [s, c_out] --------------
                for st in range(ST):
                    pv = psum.tile([P, 512], f32, tag="acc", bufs=3)
                    for ci in range(CT):
                        nc.tensor.matmul(
                            pv,
                            proj_src[:, ci, st * P:(st + 1) * P],
                            wT["wv"][:, ci, :],
                            start=(ci == 0), stop=(ci == CT - 1))
                    if uniform:
                        nc.vector.tensor_scalar(
                            v_sb[:, st], pv, rstd_c, None, MULT)
                    else:
                        nc.vector.tensor_tensor(
                            v_sb[:, st], pv, bv_bcast, ADD)

                # ---------------- attention per head ----------------
                for h in range(NH):
                    pos = [psum.tile([P, 512], f32, tag="acc", bufs=3,
                                     name=f"po{half}")
                           for half in range(2)]
                    prs = [psum.tile([1, 512], f32, tag="row", bufs=2,
                                     name=f"pr{half}")
                           for half in range(2)]
                    for kt in range(ST):
                        for half in range(2):
                            sl = slice(half * 512, (half + 1) * 512)
                            psh = psum.tile([P, 512], f32, tag="sco", bufs=3)
                            nc.tensor.matmul(
                                psh,
                                kT_sb[:, h, kt * P:(kt + 1) * P],
                                qT_sb[:, h, sl],
                                start=True, stop=True)
                            expT = exp_pool.tile([P, 512], f32r, tag="expT",
                                                 bufs=6)
                            nc.scalar.activation(expT, psh, AF.Exp,
                                                 bias=0.0, scale=SCALE)
                            nc.tensor.matmul(
                                pos[half],
                                v_sb[:, kt, h * P:(h + 1) * P],
                                expT,
                                start=(kt == 0), stop=(kt == ST - 1))
                            nc.tensor.matmul(
                                prs[half],
                                ones_col,
                                expT,
                                start=(kt == 0), stop=(kt == ST - 1))
                    for half in range(2):
                        sl = slice(half * 512, (half + 1) * 512)
                        recip = norm_pool.tile([1, S // 2], f32, tag="recip")
                        nc.vector.reciprocal(recip, prs[half])
                        rb = norm_pool.tile([P, S // 2], f32, tag="rb")
                        nc.gpsimd.partition_broadcast(rb, recip)
                        nc.vector.tensor_tensor(
                            outT_sb[:, h, sl], pos[half], rb, MULT)

                # ---------------- output projection + residual --------------
                res_src = xres_sb[b] if uniform else x_sb[b]
                for co in range(CT):
                    for half in range(2):
                        sl = slice(half * 512, (half + 1) * 512)
                        py = psum.tile([P, 512], f32, tag="acc", bufs=3)
                        nc.tensor.matmul(
                            py,
                            bo2_row[:, co * P:(co + 1) * P],
                            ones_row,
                            start=True, stop=False)
                        for ci in range(CT):
                            nc.tensor.matmul(
                                py,
                                wT["wo"][:, ci, co * P:(co + 1) * P],
                                outT_sb[:, ci, sl],
                                start=False, stop=(ci == CT - 1))
                        fin = fin_pool.tile([P, 512], f32, tag="fin")
                        nc.vector.tensor_tensor(fin, py, res_src[:, co, sl],
                                                ADD)
                        nc.sync.dma_start(out_view[b][:, co, sl], fin)

    nc.compile()
    return nc


_NC_CACHE = {}


def _get_nc(uniform=True):
    if uniform not in _NC_CACHE:
        _NC_CACHE[uniform] = _build_nc(uniform)
    return _NC_CACHE[uniform]


def run_sharded(inputs, trace=False):
    """Run on 8 cores; returns (full_output, BassKernelResults)."""
    x = np.ascontiguousarray(np.asarray(inputs["x"], dtype=np.float32))
    x = x.reshape(B, C, S)
    gnw = np.asarray(inputs["gn_weight"], np.float32)
    gnb = np.asarray(inputs["gn_bias"], np.float32)
    uniform = bool(np.all(gnw == 1.0) and np.all(gnb == 0.0))

    shared = {}
    ws = {}
    for n in ("wq", "wk", "wv", "wo"):
        wn = np.asarray(inputs[n], np.float32)
        shared[n] = np.ascontiguousarray(wn.T)
        ws[n] = wn.sum(axis=1).astype(np.float32)
    for n in ("bq", "bk", "bv", "bo"):
        shared[n] = np.ascontiguousarray(np.asarray(inputs[n], np.float32))
    shared["gn_weight"] = np.ascontiguousarray(gnw)
    shared["gn_bias"] = np.ascontiguousarray(gnb)

    def colmat(v):  # [C] -> [P, CT] with [p, t] = v[t*P + p]
        return np.asarray(v, np.float32).reshape(CT, P).T

    vecs = [shared["bq"], shared["bk"], gnw, gnb]
    if uniform:
        vecs += [shared["bv"], ws["wq"], ws["wk"], ws["wv"]]
    shared["smalls"] = np.ascontiguousarray(
        np.concatenate([colmat(v) for v in vecs], axis=1))

    in_maps = []
    for c in range(N_CORES):
        m = dict(shared)
        m["x"] = np.ascontiguousarray(x[c * BPC:(c + 1) * BPC])
        in_maps.append(m)

    nc = _get_nc(uniform)
    res = run_bass_kernel_spmd(nc, in_maps, core_ids=list(range(N_CORES)),
                               trace=trace)
    out = np.concatenate([r["out"] for r in res.results], axis=0)
    return out.reshape(B, C, H, W), res


def kernel(**inputs) -> np.ndarray:
    out, _ = run_sharded(inputs, trace=False)
    return out


# revision 33
# speedup vs baseline: 1.0038x; 1.0038x over previous
"""Trainium2 Bass kernel for nn_AttentionBlock (B=16, C=512, H=W=32, 4 heads).

Strategy: data-parallel over batch across 8 NeuronCores (2 batch elements per
core), weights replicated, no collectives.  All matmuls in float32r (full PE
rate, ~1e-4 rounding).  Attention is computed in transposed score layout
scoresT[ks, qs] so that:
  - exp() runs on ScalarE straight out of PSUM (scale fused into activation),
  - softmax denominators come from a ones-vector matmul (PE, nearly free),
  - attn @ V contracts over the partition axis natively (no transposes),
  - the output projection consumes attn-out^T directly and the residual add
    happens in the natural [C, H*W] layout of x.

Weights are transposed on the host (input prep) so no on-device transposes
are needed.

uniform=True variant (gn_weight==1, gn_bias==0, which is what setup_inputs
produces): GroupNorm is the uniform affine (x-mean)*rstd, and because softmax
over ks is invariant to per-qs shifts, the whole normalization folds out of
the critical path:
  - QKV matmuls run on RAW x (f32r straight from DMA),
  - rstd^2 folds into the exp() scale (an AP),
  - the only surviving bias term (K-side, varying over ks) is a per-partition
    vector that exp()'s bias AP applies for free; it is produced by 8 tiny
    N=1 matmuls per head,
  - rstd on the V path folds into the softmax reciprocal,
  - the V bias is constant along ks, passes through the attention average
    unchanged, and folds into the output-projection bias via 4 tiny matmuls.
So Q/K/V PSUM->SBUF moves are PURE copies (ScalarE Identity), and the stats
chain (DVE-only, Newton rsqrt) has tens of microseconds of slack.

uniform=False: fully general fallback (materializes normalized xn).
"""

import numpy as np

import concourse.bacc as bacc
import concourse.bass as bass
import concourse.mybir as mybir
import concourse.tile as tile
from concourse.bass_utils import run_bass_kernel_spmd

# Problem constants (hardcoded per harness contract).
B = 16
C = 512
H = W = 32
S = H * W            # 1024
NH = 4               # heads
HD = C // NH         # 128
P = 128              # SBUF partitions
CT = C // P          # 4 channel tiles
ST = S // P          # 8 sequence tiles
N_CORES = 8
BPC = B // N_CORES   # 2 batch elements per core
EPS = 1e-5
SCALE = float(1.0 / np.sqrt(HD))

f32 = mybir.dt.float32
f32r = mybir.dt.float32r
ADD = mybir.AluOpType.add
MULT = mybir.AluOpType.mult
SUB = mybir.AluOpType.subtract
AF = mybir.ActivationFunctionType


def _build_nc(uniform):
    nc = bacc.Bacc("TRN2", target_bir_lowering=False)

    x_d = nc.dram_tensor("x", [BPC, C, S], f32r if uniform else f32,
                         kind="ExternalInput")
    # host passes w*.T (shape [c_in, c_out])
    w_d = {n: nc.dram_tensor(n, [C, C], f32r, kind="ExternalInput")
           for n in ("wq", "wk", "wv", "wo")}
    b_d = {n: nc.dram_tensor(n, [C], f32, kind="ExternalInput")
           for n in ("bq", "bk", "bv", "bo")}
    gnw_d = nc.dram_tensor("gn_weight", [C], f32, kind="ExternalInput")
    gnb_d = nc.dram_tensor("gn_bias", [C], f32, kind="ExternalInput")
    # host-packed small vectors in SBUF layout [P, n*CT]:
    # uniform: [bq, bk, gnw, gnb, bv, wqrs, wkrs, wvrs]; else [bq, bk, gnw, gnb]
    n_small = 8 if uniform else 4
    smalls_d = nc.dram_tensor("smalls", [P, n_small * CT], f32,
                              kind="ExternalInput")
    out_d = nc.dram_tensor("out", [BPC, C, S], f32, kind="ExternalOutput")

    x_view = x_d.rearrange("b (t p) s -> b p t s", p=P)
    out_view = out_d.rearrange("b (t p) s -> b p t s", p=P)

    with tile.TileContext(nc) as tc:
        with (
            tc.tile_pool(name="persist", bufs=1) as persist,
            tc.tile_pool(name="xn_pool", bufs=2) as xn_pool,
            tc.tile_pool(name="exp_pool", bufs=6) as exp_pool,
            tc.tile_pool(name="fin_pool", bufs=4) as fin_pool,
            tc.tile_pool(name="stat_pool", bufs=2) as stat_pool,
            tc.tile_pool(name="norm_pool", bufs=3) as norm_pool,
            tc.tile_pool(name="psum", bufs=1, space="PSUM") as psum,
        ):
            # ---------------- constants / small inputs ----------------
            ones_f32 = persist.tile([P, S // 2], f32)
            nc.vector.memset(ones_f32, 1.0)
            ones_col = persist.tile([P, 1], f32r)
            nc.vector.tensor_copy(ones_col, ones_f32[:, 0:1])
            ones_row = persist.tile([1, S // 2], f32r)
            nc.vector.tensor_copy(ones_row, ones_f32[0:1, :])

            smalls_sb = persist.tile([P, n_small * CT], f32)
            nc.sync.dma_start(smalls_sb, smalls_d[:, :])
            bq_sb = smalls_sb[:, 0 * CT:1 * CT]
            bk_sb = smalls_sb[:, 1 * CT:2 * CT]
            gnw_sb = smalls_sb[:, 2 * CT:3 * CT]
            gnb_sb = smalls_sb[:, 3 * CT:4 * CT]
            bo_row = persist.tile([1, C], f32r)
            nc.sync.dma_start(
                bo_row, b_d["bo"].rearrange("(o c) -> o c", o=1).bitcast(f32r))
            if uniform:
                bv_sb = smalls_sb[:, 4 * CT:5 * CT]
                wqrs_sb = smalls_sb[:, 5 * CT:6 * CT]
                wkrs_sb = smalls_sb[:, 6 * CT:7 * CT]
                wvrs_sb = smalls_sb[:, 7 * CT:8 * CT]
            else:
                bv_bcast = persist.tile([P, C], f32)
                nc.sync.dma_start(
                    bv_bcast,
                    bass.AP(tensor=b_d["bv"], offset=0, ap=[[0, P], [1, C]]),
                )

            # ---------------- x b0 first: per c-tile chunks ----------------
            x_sb = []
            for b in range(BPC):
                xb = persist.tile([P, CT, S], f32r if uniform else f32,
                                  name=f"x_sb{b}")
                x_sb.append(xb)
            wT = {n: persist.tile([P, CT, C], f32r, name=f"{n}T")
                  for n in ("wq", "wk", "wv", "wo")}
            w_views = {n: w_d[n].rearrange("(t p) o -> p t o", p=P)
                       for n in ("wq", "wk", "wv", "wo")}
            # x_b0 dense first (stats chain completes before PE needs the
            # copies), then wq in chunks (first matmuls drip in behind)
            for t in range(CT):
                nc.sync.dma_start(x_sb[0][:, t], x_view[0][:, t])
            for t in range(CT):
                nc.sync.dma_start(wT["wq"][:, t], w_views["wq"][:, t])
            for name in ("wk", "wv", "wo"):
                nc.sync.dma_start(wT[name], w_views[name])
            nc.sync.dma_start(x_sb[1], x_view[1])
            if uniform:
                # exact-fp32 copy of x for the residual add (f32r reads are
                # rounded to ~12 mantissa bits by every engine)
                xres_sb = []
                for b in range(BPC):
                    xr = persist.tile([P, CT, S], f32, name=f"xres_sb{b}")
                    nc.sync.dma_start(xr, x_view[b].bitcast(f32))
                    xres_sb.append(xr)

            # persistent per-batch activation storage
            qT_sb = persist.tile([P, CT, S], f32r, name="qT_sb")
            kT_sb = persist.tile([P, CT, S], f32r, name="kT_sb")
            v_sb = persist.tile([P, ST, C], f32r, name="v_sb")
            outT_sb = persist.tile([P, CT, S], f32r, name="outT_sb")

            for b in range(BPC):
                # ------------- GroupNorm stats (off the critical path) ------
                stats6 = stat_pool.tile([P, CT * 2, 6], f32, tag="stats6")
                x_chunks = x_sb[b].rearrange("p t (u f) -> p (t u) f", f=512)
                for g in range(CT * 2):
                    nc.vector.bn_stats(stats6[:, g], x_chunks[:, g])
                mv = stat_pool.tile([P, 2], f32, tag="mv")
                nc.vector.bn_aggr(mv, stats6)
                msq = stat_pool.tile([P, 3], f32, tag="msq")
                nc.vector.tensor_copy(msq[:, 0:2], mv)
                nc.vector.tensor_tensor(msq[:, 2:3], mv[:, 0:1], mv[:, 0:1], MULT)
                # partition-sum via PE ones-matmul (fp32, tiny)
                red_ps = psum.tile([1, 4], f32, tag="sco", bufs=3)
                nc.tensor.matmul(red_ps[:, 0:3], ones_f32[:, 0:1], msq,
                                 start=True, stop=True)
                # sc: [mean, var+eps, avg_msq, mean^2, u, y, rstd, _]
                sc = stat_pool.tile([1, 8], f32, tag="sc")
                nc.vector.tensor_scalar_mul(sc[:, 0:3], red_ps[:, 0:3], 1.0 / P)
                nc.vector.tensor_tensor(sc[:, 3:4], sc[:, 0:1], sc[:, 0:1], MULT)
                nc.vector.tensor_tensor(sc[:, 1:2], sc[:, 1:2], sc[:, 2:3], ADD)
                nc.vector.tensor_tensor(sc[:, 1:2], sc[:, 1:2], sc[:, 3:4], SUB)
                nc.vector.tensor_scalar(sc[:, 1:2], sc[:, 1:2], EPS, None, ADD)
                u_t = sc[:, 4:5]
                nc.vector.reciprocal(u_t, sc[:, 1:2])      # u = 1/(var+eps)
                # y = rsqrt(u) = sqrt(var+eps), Newton from y0=1 (u ~= 1)
                y_t = sc[:, 5:6]
                nwt = stat_pool.tile([1, 1], f32, tag="nwt")
                nc.vector.tensor_copy(y_t, ones_f32[0:1, 0:1])
                for _ in range(3):
                    nc.vector.tensor_tensor(nwt, y_t, y_t, MULT)
                    nc.vector.tensor_tensor(nwt, nwt, u_t, MULT)
                    nc.vector.tensor_scalar(nwt, nwt, -0.5, 1.5, MULT, ADD)
                    nc.vector.tensor_tensor(y_t, y_t, nwt, MULT)
                rstd_t = sc[:, 6:7]
                nc.vector.tensor_tensor(rstd_t, u_t, y_t, MULT)

                if uniform:
                    # scal2 = [rstd, rstd*mean], broadcast via PE outer prod
                    scal2 = stat_pool.tile([1, 2], f32, tag="scal2")
                    nc.vector.tensor_copy(scal2[:, 0:1], rstd_t)
                    nc.vector.tensor_tensor(scal2[:, 1:2], rstd_t, sc[:, 0:1],
                                            MULT)
                    bc_ps = psum.tile([P, 2], f32, tag="sco", bufs=3)
                    nc.tensor.matmul(bc_ps, ones_f32[0:1, 0:P], scal2,
                                     start=True, stop=True)
                    bc = stat_pool.tile([P, 2], f32, tag="bc")
                    nc.vector.tensor_copy(bc, bc_ps)
                    rstd_c = bc[:, 0:1]
                    # b?p = bias - rstd*mean*rowsum(w)
                    bqp = stat_pool.tile([P, CT], f32, tag="bqp")
                    nc.vector.tensor_scalar(bqp, wqrs_sb, bc[:, 1:2], None, MULT)
                    nc.vector.tensor_tensor(bqp, bq_sb, bqp, SUB)
                    bkp = stat_pool.tile([P, CT], f32, tag="bkp")
                    nc.vector.tensor_scalar(bkp, wkrs_sb, bc[:, 1:2], None, MULT)
                    nc.vector.tensor_tensor(bkp, bk_sb, bkp, SUB)
                    # bvp = bv - rstd*mean*wvrs  (passes through attention)
                    tv = stat_pool.tile([P, CT], f32, tag="tv")
                    nc.vector.tensor_scalar(tv, wvrs_sb, bc[:, 1:2], None, MULT)
                    nc.vector.tensor_tensor(tv, bv_sb, tv, SUB)
                    bvp_r = stat_pool.tile([P, CT], f32r, tag="bvp_r")
                    nc.vector.tensor_copy(bvp_r, tv)
                    # delta_row[1, C] = sum_ci bvp_ci^T @ woT[ci]
                    pd = psum.tile([1, 512], f32, tag="sco", bufs=3)
                    for ci in range(CT):
                        nc.tensor.matmul(pd, bvp_r[:, ci:ci + 1],
                                         wT["wo"][:, ci, :],
                                         start=(ci == 0), stop=(ci == CT - 1))
                    bo2_row = stat_pool.tile([1, C], f32r, tag="bo2_row")
                    nc.vector.tensor_tensor(bo2_row, pd, bo_row, ADD)
                    proj_src = x_sb[b]
                else:
                    # general path: broadcast [mean, rstd]; A/Bc; xn
                    mr0 = stat_pool.tile([1, 2], f32, tag="mr0")
                    nc.vector.tensor_copy(mr0[:, 0:1], sc[:, 0:1])
                    nc.vector.tensor_copy(mr0[:, 1:2], rstd_t)
                    mr = stat_pool.tile([P, 2], f32, tag="mr")
                    nc.gpsimd.partition_broadcast(mr, mr0)
                    A = stat_pool.tile([P, CT], f32, tag="A")
                    nc.vector.tensor_scalar_mul(A, gnw_sb, mr[:, 1:2])
                    mA = stat_pool.tile([P, CT], f32, tag="mA")
                    nc.vector.tensor_scalar_mul(mA, A, mr[:, 0:1])
                    Bc = stat_pool.tile([P, CT], f32, tag="Bc")
                    nc.vector.tensor_tensor(Bc, gnb_sb, mA, SUB)
                    xn = xn_pool.tile([P, CT, S], f32r, tag="xn")
                    for t in range(CT):
                        nc.vector.tensor_scalar(
                            xn[:, t], x_sb[b][:, t], A[:, t:t + 1],
                            Bc[:, t:t + 1], MULT, ADD)
                    bo2_row = bo_row
                    proj_src = xn

                # ---------------- Q/K projections -> qT/kT [c_out, s] -------
                for (wname, dst, bias_t) in (("wq", qT_sb, bq_sb),
                                             ("wk", kT_sb, bk_sb)):
                    for co in range(CT):
                        for half in range(2):
                            sl = slice(half * 512, (half + 1) * 512)
                            pq = psum.tile([P, 512], f32, tag="acc", bufs=3)
                            for ci in range(CT):
                                nc.tensor.matmul(
                                    pq,
                                    wT[wname][:, ci, co * P:(co + 1) * P],
                                    proj_src[:, ci, sl],
                                    start=(ci == 0), stop=(ci == CT - 1))
                            if uniform:
                                bp = bqp if wname == "wq" else bkp
                                nc.vector.tensor_scalar(
                                    dst[:, co, sl], pq, rstd_c,
                                    bp[:, co:co + 1], MULT, ADD)
                            else:
                                nc.scalar.activation(
                                    dst[:, co, sl], pq, AF.Identity,
                                    bias=bias_t[:, co:co + 1], scale=1.0)

                # ---------------- V projection -> v [s, c_out] --------------
                for st in range(ST):
                    pv = psum.tile([P, 512], f32, tag="acc", bufs=3)
                    for ci in range(CT):
                        nc.tensor.matmul(
                            pv,
                            proj_src[:, ci, st * P:(st + 1) * P],
                            wT["wv"][:, ci, :],
                            start=(ci == 0), stop=(ci == CT - 1))
                    if uniform:
                        nc.vector.tensor_scalar(
                            v_sb[:, st], pv, rstd_c, None, MULT)
                    else:
                        nc.vector.tensor_tensor(
                            v_sb[:, st], pv, bv_bcast, ADD)

                # ---------------- attention per head ----------------
                for h in range(NH):
                    pos = [psum.tile([P, 512], f32, tag="acc", bufs=3,
                                     name=f"po{half}")
                           for half in range(2)]
                    prs = [psum.tile([1, 512], f32, tag="row", bufs=2,
                                     name=f"pr{half}")
                           for half in range(2)]
                    for kt in range(ST):
                        for half in range(2):
                            sl = slice(half * 512, (half + 1) * 512)
                            psh = psum.tile([P, 512], f32, tag="sco", bufs=3)
                            nc.tensor.matmul(
                                psh,
                                kT_sb[:, h, kt * P:(kt + 1) * P],
                                qT_sb[:, h, sl],
                                start=True, stop=True)
                            expT = exp_pool.tile([P, 512], f32r, tag="expT",
                                                 bufs=6)
                            nc.scalar.activation(expT, psh, AF.Exp,
                                                 bias=0.0, scale=SCALE)
                            nc.tensor.matmul(
                                pos[half],
                                v_sb[:, kt, h * P:(h + 1) * P],
                                expT,
                                start=(kt == 0), stop=(kt == ST - 1))
                            nc.tensor.matmul(
                                prs[half],
                                ones_col,
                                expT,
                                start=(kt == 0), stop=(kt == ST - 1))
                    for half in range(2):
                        sl = slice(half * 512, (half + 1) * 512)
                        recip = norm_pool.tile([1, S // 2], f32, tag="recip")
                        nc.vector.reciprocal(recip, prs[half])
                        rb = norm_pool.tile([P, S // 2], f32, tag="rb")
                        nc.gpsimd.partition_broadcast(rb, recip)
                        nc.vector.tensor_tensor(
                            outT_sb[:, h, sl], pos[half], rb, MULT)

                # ---------------- output projection + residual --------------
                res_src = xres_sb[b] if uniform else x_sb[b]
                for co in range(CT):
                    for half in range(2):
                        sl = slice(half * 512, (half + 1) * 512)
                        py = psum.tile([P, 512], f32, tag="acc", bufs=3)
                        nc.tensor.matmul(
                            py,
                            bo2_row[:, co * P:(co + 1) * P],
                            ones_row,
                            start=True, stop=False)
                        for ci in range(CT):
                            nc.tensor.matmul(
                                py,
                                wT["wo"][:, ci, co * P:(co + 1) * P],
                                outT_sb[:, ci, sl],
                                start=False, stop=(ci == CT - 1))
                        fin = fin_pool.tile([P, 512], f32, tag="fin")
                        nc.vector.tensor_tensor(fin, py, res_src[:, co, sl],
                                                ADD)
                        nc.sync.dma_start(out_view[b][:, co, sl], fin)

    nc.compile()
    return nc


_NC_CACHE = {}


def _get_nc(uniform=True):
    if uniform not in _NC_CACHE:
        _NC_CACHE[uniform] = _build_nc(uniform)
    return _NC_CACHE[uniform]


def run_sharded(inputs, trace=False):
    """Run on 8 cores; returns (full_output, BassKernelResults)."""
    x = np.ascontiguousarray(np.asarray(inputs["x"], dtype=np.float32))
    x = x.reshape(B, C, S)
    gnw = np.asarray(inputs["gn_weight"], np.float32)
    gnb = np.asarray(inputs["gn_bias"], np.float32)
    uniform = bool(np.all(gnw == 1.0) and np.all(gnb == 0.0))

    shared = {}
    ws = {}
    for n in ("wq", "wk", "wv", "wo"):
        wn = np.asarray(inputs[n], np.float32)
        shared[n] = np.ascontiguousarray(wn.T)
        ws[n] = wn.sum(axis=1).astype(np.float32)
    for n in ("bq", "bk", "bv", "bo"):
        shared[n] = np.ascontiguousarray(np.asarray(inputs[n], np.float32))
    shared["gn_weight"] = np.ascontiguousarray(gnw)
    shared["gn_bias"] = np.ascontiguousarray(gnb)

    def colmat(v):  # [C] -> [P, CT] with [p, t] = v[t*P + p]
        return np.asarray(v, np.float32).reshape(CT, P).T

    vecs = [shared["bq"], shared["bk"], gnw, gnb]
    if uniform:
        vecs += [shared["bv"], ws["wq"], ws["wk"], ws["wv"]]
    shared["smalls"] = np.ascontiguousarray(
        np.concatenate([colmat(v) for v in vecs], axis=1))

    in_maps = []
    for c in range(N_CORES):
        m = dict(shared)
        m["x"] = np.ascontiguousarray(x[c * BPC:(c + 1) * BPC])
        in_maps.append(m)

    nc = _get_nc(uniform)
    res = run_bass_kernel_spmd(nc, in_maps, core_ids=list(range(N_CORES)),
                               trace=trace)
    out = np.concatenate([r["out"] for r in res.results], axis=0)
    return out.reshape(B, C, H, W), res


def kernel(**inputs) -> np.ndarray:
    out, _ = run_sharded(inputs, trace=False)
    return out
